# revision 16
# baseline (speedup 1.0000x reference)
"""Nystrom attention TRN2 kernel.

Sharding: 8 cores = 4 batches x 2 head-groups (4 heads each). Each core
computes its batch's attention for its heads plus a partial output
projection; host sums the two partials per batch (bias folded into the
even core's partial).

Per-core pipeline (all matmuls fp32r on the PE):
  phase 1: qkv^T = w_sub^T @ x^T (x transposed on-device via PE),
           landmark sums folded into the evacuations, q/k staged to DRAM
           transposed, v staged natural (zero-padded, ones-augmented).
  phase 2A (per head): attn2 via S2 row-softmax, pinv by Newton-Schulz
           tracking (z, z^T) with the symmetric product W = z @ attn2,
           attn3 @ v streamed in n-chunks with a ones column giving the
           softmax denominators, Z = attn2_inv @ out3.
  phase 2B: E^T = exp(S1^T) per token tile, ones-augmented Z matmul gives
           attention rows + denominators, depthwise conv as Toeplitz
           matmuls, fused output projection + bias.
"""

import sys

if "/opt/trn_rl_repo" not in sys.path:
    sys.path.insert(0, "/opt/trn_rl_repo")

import numpy as np

import concourse.bass as bass
import concourse.mybir as mybir
from concourse.bass_utils import run_bass_kernel_spmd
from concourse.masks import make_identity
from concourse.tile import TileContext

# Cache compiled NEFFs on disk keyed by BIR hash: walrus on this program
# takes minutes, which can outlive the axon backend's connection timeout.
# With the cache, PJRT's automatic retries (and later runs) load instantly.
import hashlib as _hashlib
import os as _os
import shutil as _shutil

_NEFF_CACHE_DIR = "/tmp/bass_neff_cache"


def _install_neff_cache():
    import concourse.bass2jax as _b2j
    import concourse.bass_utils as _bu

    if getattr(_b2j, "_ant_neff_cache_installed", False):
        return
    _orig = _bu.compile_bir_kernel

    def cached_compile_bir_kernel(bir_json, tmpdir, neff_name="file.neff"):
        key = _hashlib.sha256(
            bir_json if isinstance(bir_json, bytes) else bir_json.encode()
        ).hexdigest()[:32]
        _os.makedirs(_NEFF_CACHE_DIR, exist_ok=True)
        cpath = _os.path.join(_NEFF_CACHE_DIR, f"{key}.neff")
        dst = _os.path.join(tmpdir, neff_name)
        if _os.path.exists(cpath):
            _shutil.copyfile(cpath, dst)
            return dst
        neff = _orig(bir_json, tmpdir, neff_name=neff_name)
        try:
            _shutil.copyfile(neff, cpath + ".tmp")
            _os.replace(cpath + ".tmp", cpath)
        except OSError:
            pass
        return neff

    _b2j.compile_bir_kernel = cached_compile_bir_kernel
    _b2j._ant_neff_cache_installed = True


_install_neff_cache()

F32 = mybir.dt.float32
F32R = mybir.dt.float32r
EXP = mybir.ActivationFunctionType.Exp
ADD = mybir.AluOpType.add
DIV = mybir.AluOpType.divide
X = mybir.AxisListType.X

N, DIM, DH, M, NH = 8192, 512, 64, 256, 4  # tokens, dim, head_dim, landmarks, heads/core
SEG = N // M          # tokens per landmark = 32
NJ = N // 512         # 16 token-512 tiles
NC = N // 128         # 64 token-128 chunks
PITERS = 6
KER = 33


def _r(ap):
    return ap.bitcast(F32R)


def _pbcast(ap, p, free):
    """Partition-broadcast AP for DMA: read ap's partition 0 across p rows."""
    return bass.AP(tensor=ap.tensor, offset=ap.offset, ap=[[0, p]] + free)




def _copy(eng, out, in_):
    if eng is None or eng.__class__.__name__ == "BassVectorEngine":
        eng = eng
    if hasattr(eng, "tensor_copy"):
        eng.tensor_copy(out=out, in_=in_)
    else:
        eng.copy(out=out, in_=in_)


def build_program():
    nc = bass.Bass("TRN2", target_bir_lowering=False, debug=False, num_devices=8)

    x_d = nc.dram_tensor("x", [N, DIM], F32, kind="ExternalInput").ap()
    w_d = nc.dram_tensor("w_sub", [DIM, 3 * NH * DH], F32, kind="ExternalInput").ap()
    wout_d = nc.dram_tensor("w_out_sub", [NH * DH, DIM], F32, kind="ExternalInput").ap()
    bias_d = nc.dram_tensor("bias", [DIM], F32, kind="ExternalInput").ap()
    tconv_d = nc.dram_tensor("tconv", [NH, 288, 256], F32, kind="ExternalInput").ap()
    out_d = nc.dram_tensor("out", [N, DIM], F32, kind="ExternalOutput").ap()
    qT_d = nc.dram_tensor("qT_s", [NH, DH, N], F32).ap()
    kT_d = nc.dram_tensor("kT_s", [NH, DH, N], F32).ap()
    v_d = nc.dram_tensor("v_s", [NH, N + 32, DH + 1], F32).ap()
    dscr_d = nc.dram_tensor("d_scr", [NJ, NH, 512], F32).ap()
    d3scr_d = nc.dram_tensor("d3_scr", [NH, M], F32).ap()

    with nc.allow_low_precision(reason="fp32r staging, fp32 PSUM accumulate"), \
         TileContext(nc) as tc:
        with (
            tc.tile_pool(name="consts", bufs=1) as consts,
            tc.tile_pool(name="zap", bufs=4) as zap,
        ):
            ident = consts.tile([128, 128], F32, tag="ident", name="ident")
            make_identity(nc, ident)
            ones128 = consts.tile([128, 1], F32, tag="ones128", name="ones128")
            nc.vector.memset(ones128, 1.0)
            ones_full = consts.tile([128, 128], F32, tag="ones_full", name="ones_full")
            nc.vector.memset(ones_full, 1.0)
            ztile = consts.tile([128, DH + 1], F32, tag="ztile", name="ztile")
            nc.vector.memset(ztile, 0.0)
            nc.vector.memset(ztile[:, DH : DH + 1], 1.0)

            w_sb = consts.tile([128, 4, 768], F32, tag="w", name="w")
            nc.sync.dma_start(out=_r(w_sb), in_=_r(w_d.rearrange("(c p) o -> p c o", p=128)))
            wout_sb = consts.tile([128, 2, DIM], F32, tag="wout", name="wout")
            nc.sync.dma_start(
                out=_r(wout_sb), in_=_r(wout_d.rearrange("(c p) o -> p c o", p=128))
            )
            bias_rep = consts.tile([128, DIM], F32, tag="bias", name="bias")
            nc.sync.dma_start(
                out=bias_rep, in_=_pbcast(bias_d, 128, [[1, DIM]])
            )
            tconv_sb = consts.tile([128, NH, 3, 256], F32, tag="tconv", name="tconv")
            for h in range(NH):
                for ck in range(3):
                    rows = 128 if ck < 2 else 32
                    nc.sync.dma_start(
                        out=_r(tconv_sb[0:rows, h, ck, :]),
                        in_=_r(tconv_d[h, 128 * ck : 128 * ck + rows, :]),
                    )
            # aI constant tiles for the pinv iteration (a = 7, 15, 13/4)
            eyes = {}
            for nm, a in (("e7", 7.0), ("e15", 15.0), ("e325", 3.25)):
                t = consts.tile([128, 2, M], F32, tag=nm, name=nm)
                nc.vector.memset(t, 0.0)
                for r in range(2):
                    nc.scalar.activation(
                        out=t[:, r, 128 * r : 128 * r + 128],
                        in_=ident,
                        func=mybir.ActivationFunctionType.Copy,
                        scale=a,
                    )
                eyes[nm] = t

            # landmark accumulators (x2 head-pairs for q and k)
            lands = {}
            for nm in ("ql0", "ql1", "kl0", "kl1"):
                lands[nm] = consts.tile([128, M], F32, tag=nm, name=nm)

            # v_d pad init: zero edge rows (ones in col DH) + ones column
            onecol = ztile[:, DH : DH + 1]
            for h in range(NH):
                nc.sync.dma_start(out=_r(v_d[h, 0:16, :]), in_=_r(ztile[0:16, :]))
                nc.sync.dma_start(out=_r(v_d[h, N + 16 : N + 32, :]), in_=_r(ztile[0:16, :]))
                nc.sync.dma_start(
                    out=_r(v_d[h, 16 : 16 + N, DH : DH + 1].rearrange(
                        "(t p) c -> p t c", p=128
                    )),
                    in_=_r(bass.AP(
                        tensor=onecol.tensor,
                        offset=onecol.offset,
                        ap=[list(onecol.ap[0]), [0, N // 128], [0, 1]],
                    )),
                )

            # ---------------- phase 1: QKV ----------------
            with (
                tc.tile_pool(name="p1sb", bufs=2) as p1sb,
                tc.tile_pool(name="p1ev", bufs=3) as p1ev,
                tc.tile_pool(name="p1ps", bufs=3, space="PSUM") as p1ps,
                tc.tile_pool(name="p1ps2", bufs=2, space="PSUM") as p1ps2,
            ):
                for j in range(NJ):
                    xt = p1sb.tile([128, 4, 512], F32, tag="xt", name="xt")
                    nc.sync.dma_start(
                        out=xt,
                        in_=x_d[512 * j : 512 * (j + 1), :].rearrange(
                            "(t p) d -> p t d", p=128
                        ),
                    )
                    xT = p1sb.tile([128, 4, 512], F32, tag="xT", name="xT")
                    for t in range(4):
                        for c in range(4):
                            ps = p1ps.tile([128, 128], F32, tag="ptr", name="ptr")
                            nc.tensor.transpose(
                                ps, xt[:, t, 128 * c : 128 * (c + 1)], ident
                            )
                            eng = nc.vector if (t + c) % 2 == 0 else nc.scalar
                            _copy(eng, _r(xT[:, c, 128 * t : 128 * (t + 1)]), ps)
                    for oc in range(6):
                        ps = p1ps2.tile([128, 512], F32, tag="pqkv", name="pqkv")
                        for c in range(4):
                            nc.tensor.matmul(
                                ps,
                                _r(w_sb[:, c, 128 * oc : 128 * (oc + 1)]),
                                _r(xT[:, c, :]),
                                start=(c == 0),
                                stop=(c == 3),
                            )
                        sb = p1ev.tile([128, 512], F32, tag="ev", name="ev")
                        eng = nc.vector if oc % 2 == 0 else nc.scalar
                        _copy(eng, _r(sb), ps)
                        if oc < 4:
                            isq = oc < 2
                            dst = qT_d if isq else kT_d
                            pair = oc % 2
                            for half in range(2):
                                nc.sync.dma_start(
                                    out=_r(dst[
                                        2 * pair + half, :, 512 * j : 512 * (j + 1)
                                    ]),
                                    in_=_r(sb[64 * half : 64 * (half + 1), :]),
                                )
                            land = lands[("ql" if isq else "kl") + str(pair)]
                            nc.vector.reduce_sum(
                                out=_r(land[:, 16 * j : 16 * (j + 1)]),
                                in_=sb.rearrange("p (s l) -> p s l", l=SEG),
                                axis=X,
                            )
                        else:
                            pair = oc - 4
                            for t in range(4):
                                pst = p1ps.tile([128, 128], F32, tag="ptr", name="ptr")
                                nc.tensor.transpose(
                                    pst, sb[:, 128 * t : 128 * (t + 1)], ident
                                )
                                vtmp = p1ev.tile([128, 128], F32, tag="vtmp", name="vtmp")
                                eng = nc.vector if t % 2 == 0 else nc.scalar
                                _copy(eng, _r(vtmp), pst)
                                row = 16 + 512 * j + 128 * t
                                for half in range(2):
                                    nc.sync.dma_start(
                                        out=_r(v_d[
                                            2 * pair + half, row : row + 128, 0:DH
                                        ]),
                                        in_=_r(vtmp[:, 64 * half : 64 * (half + 1)]),
                                    )

            # scale landmark sums -> means
            for nm in lands:
                nc.vector.tensor_scalar_mul(
                    out=_r(lands[nm]), in0=lands[nm], scalar1=1.0 / SEG
                )
            # per-head landmark tiles at base partition 0 (matmul operands
            # must share base_partition)
            qlts, klts = [], []
            for h in range(NH):
                qlh = consts.tile([DH, M], F32, tag=f"qlh{h}", name=f"qlh{h}")
                klh = consts.tile([DH, M], F32, tag=f"klh{h}", name=f"klh{h}")
                nc.sync.dma_start(
                    out=_r(qlh),
                    in_=_r(lands["ql" + str(h // 2)][
                        64 * (h % 2) : 64 * (h % 2) + 64, :
                    ]),
                )
                nc.sync.dma_start(
                    out=_r(klh),
                    in_=_r(lands["kl" + str(h // 2)][
                        64 * (h % 2) : 64 * (h % 2) + 64, :
                    ]),
                )
                qlts.append(qlh)
                klts.append(klh)

            # ---------------- phase 2A: per-head attn2/pinv/out3 ----------------
            zaugs = []
            with (
                tc.tile_pool(name="a2sb", bufs=2) as a2sb,
                tc.tile_pool(name="pvsb", bufs=2) as pvsb,
                tc.tile_pool(name="ktp", bufs=2) as ktp,
                tc.tile_pool(name="sml", bufs=4) as sml,
                tc.tile_pool(name="a2ps", bufs=3, space="PSUM") as a2ps,
                tc.tile_pool(name="o3ps", bufs=1, space="PSUM") as o3ps,
                tc.tile_pool(name="s3ps", bufs=2, space="PSUM") as s3ps,
            ):
                for h in range(NH):
                    qlt, klt = qlts[h], klts[h]

                    # S2 natural -> E2 -> A2 (row softmax, no max subtraction)
                    e2 = a2sb.tile([128, 2, M], F32, tag="e2", name="e2")
                    for r in range(2):
                        ps = a2ps.tile([128, M], F32, tag="mm", name="mm")
                        nc.tensor.matmul(
                            ps,
                            _r(qlt[:, 128 * r : 128 * (r + 1)]),
                            _r(klt),
                            start=True,
                            stop=True,
                        )
                        nc.scalar.activation(out=e2[:, r, :], in_=ps, func=EXP)
                    d2 = sml.tile([128, 2], F32, tag="d2", name="d2")
                    for r in range(2):
                        nc.vector.reduce_sum(
                            out=d2[:, r : r + 1], in_=e2[:, r, :], axis=X
                        )
                    d2i = sml.tile([128, 2], F32, tag="d2i", name="d2i")
                    nc.vector.reciprocal(out=d2i, in_=d2)
                    a2 = a2sb.tile([128, 2, M], F32, tag="a2", name="a2")
                    for r in range(2):
                        nc.vector.tensor_scalar_mul(
                            out=_r(a2[:, r, :]), in0=e2[:, r, :],
                            scalar1=d2i[:, r : r + 1],
                        )
                    # B = A2^T
                    bmat = a2sb.tile([128, 2, M], F32, tag="bmat", name="bmat")
                    for rc in range(2):
                        for cc in range(2):
                            ps = a2ps.tile([128, 128], F32, tag="sm", name="sm", bufs=1)
                            nc.tensor.transpose(
                                ps, a2[:, cc, 128 * rc : 128 * (rc + 1)], ident
                            )
                            eng = nc.vector if (rc + cc) % 2 == 0 else nc.scalar
                            _copy(eng, _r(bmat[:, rc, 128 * cc : 128 * (cc + 1)]), ps)
                    # z0 scale = 1 / max(colsum(A2)); colsums replicated on
                    # all partitions via an all-ones stationary operand
                    csp = a2ps.tile([128, M], F32, tag="sm", name="sm", bufs=1)
                    for r in range(2):
                        nc.tensor.matmul(
                            csp, _r(ones_full), _r(a2[:, r, :]),
                            start=(r == 0), stop=(r == 1),
                        )
                    cs = sml.tile([128, M], F32, tag="cssb", name="cssb")
                    nc.vector.tensor_copy(out=cs, in_=csp)
                    mx = sml.tile([128, 1], F32, tag="mx", name="mx")
                    nc.vector.reduce_max(out=mx, in_=cs, axis=X)
                    mxr = sml.tile([128, 1], F32, tag="mxr", name="mxr")
                    nc.vector.reciprocal(out=mxr, in_=mx)

                    z = a2sb.tile([128, 2, M], F32, tag="z", name="z")
                    zt = a2sb.tile([128, 2, M], F32, tag="zt", name="zt")
                    for r in range(2):
                        nc.vector.tensor_scalar_mul(
                            out=_r(z[:, r, :]), in0=bmat[:, r, :], scalar1=mxr[:, 0:1]
                        )
                        nc.scalar.activation(
                            out=_r(zt[:, r, :]), in_=a2[:, r, :],
                            func=mybir.ActivationFunctionType.Copy,
                            scale=mxr[:, 0:1],
                        )

                    def mm22(lhsT_sb, rhs_sb):
                        """[M,M] product: out[r] = sum_kc lhsT[kc][:,r].T @ rhs[kc]."""
                        outs = []
                        for r in range(2):
                            ps = a2ps.tile([128, M], F32, tag="mm", name="mm")
                            for kc in range(2):
                                nc.tensor.matmul(
                                    ps,
                                    _r(lhsT_sb[:, kc, 128 * r : 128 * (r + 1)]),
                                    _r(rhs_sb[:, kc, :]),
                                    start=(kc == 0),
                                    stop=(kc == 1),
                                )
                            outs.append(ps)
                        return outs

                    for it in range(PITERS):
                        wps = mm22(zt, a2)
                        wsb = a2sb.tile([128, 2, M], F32, tag="wsb", name="wsb")
                        for r in range(2):
                            eng = nc.vector if r == 0 else nc.scalar
                            _copy(eng, _r(wsb[:, r, :]), wps[r])
                        t1 = a2sb.tile([128, 2, M], F32, tag="t1", name="t1")
                        for r in range(2):
                            nc.vector.tensor_tensor(
                                out=_r(t1[:, r, :]), in0=eyes["e7"][:, r, :],
                                in1=wsb[:, r, :], op=mybir.AluOpType.subtract,
                            )
                        t2ps = mm22(wsb, t1)
                        t3 = a2sb.tile([128, 2, M], F32, tag="t3", name="t3")
                        for r in range(2):
                            nc.vector.tensor_tensor(
                                out=_r(t3[:, r, :]), in0=eyes["e15"][:, r, :],
                                in1=t2ps[r], op=mybir.AluOpType.subtract,
                            )
                        t4ps = mm22(wsb, t3)
                        pmat = a2sb.tile([128, 2, M], F32, tag="pmat", name="pmat")
                        for r in range(2):
                            nc.vector.scalar_tensor_tensor(
                                out=_r(pmat[:, r, :]), in0=t4ps[r], scalar=-0.25,
                                in1=eyes["e325"][:, r, :],
                                op0=mybir.AluOpType.mult, op1=ADD,
                            )
                        znps = mm22(pmat, z)
                        ztnps = mm22(z, pmat)
                        z = a2sb.tile([128, 2, M], F32, tag="z", name="z")
                        zt = a2sb.tile([128, 2, M], F32, tag="zt", name="zt")
                        for r in range(2):
                            nc.vector.tensor_copy(out=_r(z[:, r, :]), in_=znps[r])
                            nc.scalar.copy(out=_r(zt[:, r, :]), in_=ztnps[r])

                    # out3 streaming over n-chunks
                    kt = ktp.tile([DH, N], F32, tag="kt", name="kt")
                    nc.sync.dma_start(out=_r(kt), in_=_r(kT_d[h]))
                    o3p = o3ps.tile([DH + 1, M], F32, tag="o3", name="o3")
                    for i in range(NC):
                        vch = pvsb.tile([128, DH + 1], F32, tag="vch", name="vch")
                        nc.sync.dma_start(
                            out=_r(vch),
                            in_=_r(v_d[h, 16 + 128 * i : 16 + 128 * (i + 1), :]),
                        )
                        ps3 = s3ps.tile([128, M], F32, tag="ps3", name="ps3")
                        nc.tensor.matmul(
                            ps3,
                            _r(kt[:, 128 * i : 128 * (i + 1)]),
                            _r(qlt),
                            start=True,
                            stop=True,
                        )
                        p3 = pvsb.tile([128, M], F32, tag="p3", name="p3")
                        nc.scalar.activation(out=_r(p3), in_=ps3, func=EXP)
                        nc.tensor.matmul(
                            o3p, _r(vch), _r(p3), start=(i == 0), stop=(i == NC - 1)
                        )
                    o3sb = sml.tile([DH + 1, M], F32, tag="o3sb", name="o3sb")
                    nc.vector.tensor_copy(out=o3sb, in_=o3p)
                    d3i = sml.tile([1, M], F32, tag="d3i", name="d3i")
                    nc.vector.reciprocal(out=d3i, in_=o3sb[DH : DH + 1, :])
                    d3it = sml.tile([128, 2], F32, tag="d3it", name="d3it")
                    nc.sync.dma_start(out=d3scr_d[h, :], in_=d3i)
                    nc.sync.dma_start(
                        out=d3it,
                        in_=bass.AP(
                            tensor=d3scr_d.tensor,
                            offset=d3scr_d.offset + h * M,
                            ap=[[1, 128], [128, 2]],
                        ),
                    )
                    o3n = sml.tile([128, 2, DH], F32, tag="o3n", name="o3n")
                    for c in range(2):
                        ps = a2ps.tile([128, DH], F32, tag="sm", name="sm", bufs=1)
                        nc.tensor.transpose(
                            ps, o3sb[0:DH, 128 * c : 128 * (c + 1)], ident[0:DH, 0:DH]
                        )
                        nc.vector.tensor_scalar_mul(
                            out=_r(o3n[:, c, :]), in0=ps, scalar1=d3it[:, c : c + 1]
                        )
                    zaug = zap.tile([128, 2, DH + 1], F32, tag="zaug", name="zaug")
                    nc.vector.memset(zaug[:, :, DH : DH + 1], 1.0)
                    for r in range(2):
                        ps = a2ps.tile([128, DH], F32, tag="sm", name="sm", bufs=1)
                        for kc in range(2):
                            nc.tensor.matmul(
                                ps,
                                _r(zt[:, kc, 128 * r : 128 * (r + 1)]),
                                _r(o3n[:, kc, :]),
                                start=(kc == 0),
                                stop=(kc == 1),
                            )
                        nc.scalar.copy(out=_r(zaug[:, r, 0:DH]), in_=ps)
                    zaugs.append(zaug)

            # ---------------- phase 2B: E^T, conv, projection ----------------
            with (
                tc.tile_pool(name="ocp", bufs=1) as ocp,
                tc.tile_pool(name="bsb", bufs=3) as bsb,
                tc.tile_pool(name="bsm", bufs=4) as bsm,
                tc.tile_pool(name="eps", bufs=2, space="PSUM") as eps,
                tc.tile_pool(name="aps", bufs=2, space="PSUM") as aps,
                tc.tile_pool(name="cps", bufs=2, space="PSUM") as cps,
                tc.tile_pool(name="pps", bufs=2, space="PSUM") as pps,
            ):
                oc0 = ocp.tile([128, N], F32, tag="oc0", name="oc0")
                oc1 = ocp.tile([128, N], F32, tag="oc1", name="oc1")
                ocs = [oc0, oc1]
                for j in range(NJ):
                    for h in range(NH):
                        klt = klts[h]
                        qsl = bsb.tile([DH, 512], F32, tag="qsl", name="qsl")
                        nc.sync.dma_start(
                            out=_r(qsl), in_=_r(qT_d[h, :, 512 * j : 512 * (j + 1)])
                        )
                        et = bsb.tile([128, 2, 512], F32, tag="et", name="et")
                        for c in range(2):
                            pse = eps.tile([128, 512], F32, tag="pse", name="pse")
                            nc.tensor.matmul(
                                pse,
                                _r(klt[:, 128 * c : 128 * (c + 1)]),
                                _r(qsl),
                                start=True,
                                stop=True,
                            )
                            nc.scalar.activation(out=_r(et[:, c, :]), in_=pse, func=EXP)
                        pa = aps.tile([DH + 1, 512], F32, tag="mm", name="mm")
                        for c in range(2):
                            nc.tensor.matmul(
                                pa,
                                _r(zaugs[h][:, c, :]),
                                _r(et[:, c, :]),
                                start=(c == 0),
                                stop=(c == 1),
                            )
                        # conv: padded v rows [512j, 512j+544)
                        vw = bsb.tile([128, 5, DH + 1], F32, tag="vw", name="vw")
                        nc.sync.dma_start(
                            out=_r(vw[:, 0:4, :]),
                            in_=_r(v_d[h, 512 * j : 512 * (j + 1), :].rearrange(
                                "(t p) c -> p t c", p=128
                            )),
                        )
                        nc.sync.dma_start(
                            out=_r(vw[0:32, 4, :]),
                            in_=_r(v_d[h, 512 * (j + 1) : 512 * (j + 1) + 32, :]),
                        )
                        pc = cps.tile([DH, 512], F32, tag="pc", name="pc")
                        for s in range(2):
                            for ck in range(3):
                                ti = 2 * s + ck
                                if ck < 2:
                                    lhs = vw[:, ti, 0:DH]
                                    rhs = tconv_sb[:, h, ck, :]
                                else:
                                    lhs = vw[0:32, ti, 0:DH]
                                    rhs = tconv_sb[0:32, h, 2, :]
                                nc.tensor.matmul(
                                    pc[:, 256 * s : 256 * (s + 1)],
                                    _r(lhs),
                                    _r(rhs),
                                    start=(ck == 0),
                                    stop=(ck == 2),
                                )
                        dsb = bsm.tile([1, 512], F32, tag="dsb", name="dsb")
                        nc.scalar.copy(out=dsb, in_=pa[DH : DH + 1, :])
                        dsbi = bsm.tile([1, 512], F32, tag="dsbi", name="dsbi")
                        nc.vector.reciprocal(out=dsbi, in_=dsb)
                        nc.sync.dma_start(out=dscr_d[j, h, :], in_=dsbi)
                        drep = bsm.tile([DH, 512], F32, tag="drep", name="drep")
                        nc.sync.dma_start(
                            out=drep, in_=_pbcast(dscr_d[j, h, :], DH, [[1, 512]])
                        )
                        oslice = ocs[h // 2][
                            64 * (h % 2) : 64 * (h % 2) + 64, 512 * j : 512 * (j + 1)
                        ]
                        nc.vector.tensor_tensor(
                            out=_r(oslice), in0=pa[0:DH, :], in1=drep,
                            op=mybir.AluOpType.mult,
                        )
                        nc.vector.tensor_tensor(
                            out=_r(oslice), in0=pc, in1=oslice, op=ADD
                        )
                    for t in range(4):
                        pp = pps.tile([128, DIM], F32, tag="pp", name="pp")
                        col = 512 * j + 128 * t
                        for c in range(2):
                            nc.tensor.matmul(
                                pp,
                                _r(ocs[c][:, col : col + 128]),
                                _r(wout_sb[:, c, :]),
                                start=(c == 0),
                                stop=(c == 1),
                            )
                        osb = bsb.tile([128, DIM], F32, tag="osb", name="osb")
                        nc.vector.tensor_tensor(
                            out=osb, in0=pp, in1=bias_rep, op=ADD
                        )
                        nc.sync.dma_start(out=out_d[col : col + 128, :], in_=osb)

    split_excess_waits(nc)
    return nc


def split_excess_waits(nc, max_waits: int = 1):
    """walrus in this image accepts a single sync-wait per instruction;
    move extra waits onto same-engine NoOps inserted just before."""
    n_split = 0
    for f in nc.m.functions:
        for blk in f.blocks:
            insts = blk.instructions
            new_insts = []
            for inst in insts:
                si = getattr(inst, "sync_info", None)
                if si is not None and len(si.on_wait) > max_waits:
                    waits = list(si.on_wait)
                    for w in waits[max_waits:]:
                        nop = mybir.InstNoOp(
                            name=f"{inst.name}-waitsplit-{n_split}",
                            sync_info=mybir.SyncInfo(on_wait=[w], on_update=[]),
                            bass_nofuse=True,
                            engine=inst.engine,
                        )
                        nop.bass_scheduled_proc = inst.bass_scheduled_proc
                        nop.bass_scheduled_tick = inst.bass_scheduled_tick
                        new_insts.append(nop)
                        n_split += 1
                    inst.sync_info = mybir.SyncInfo(
                        on_wait=waits[:max_waits], on_update=list(si.on_update)
                    )
                new_insts.append(inst)
            if len(new_insts) != len(insts):
                blk.instructions = new_insts
    return n_split


def make_core_inputs(core, x, w_qkv, w_out, b_out, res_kernel):
    b, g = core // 2, core % 2
    cols = np.concatenate(
        [np.arange(h * DH, (h + 1) * DH) for h in range(4 * g, 4 * g + 4)]
    )
    scale = DH ** -0.5
    w_sub = np.ascontiguousarray(
        np.concatenate(
            [w_qkv[:, cols] * scale, w_qkv[:, 512 + cols], w_qkv[:, 1024 + cols]],
            axis=1,
        ),
        dtype=np.float32,
    )
    w_out_sub = np.ascontiguousarray(w_out[g * 256 : (g + 1) * 256], dtype=np.float32)
    bias = (b_out if core % 2 == 0 else np.zeros_like(b_out)).astype(np.float32)
    ker = res_kernel[:, 0, :, 0]
    tconv = np.zeros((NH, 288, 256), np.float32)
    idx = np.arange(256)
    for i, h in enumerate(range(4 * g, 4 * g + 4)):
        for jj in range(KER):
            tconv[i, idx + jj, idx] = ker[h, jj]
    return {
        "x": np.ascontiguousarray(x[b], dtype=np.float32),
        "w_sub": w_sub,
        "w_out_sub": w_out_sub,
        "bias": bias,
        "tconv": tconv,
    }


_NC = None


def kernel(x, w_qkv, w_out, b_out, res_kernel):
    global _NC
    x = np.asarray(x, dtype=np.float32)
    w_qkv = np.asarray(w_qkv, dtype=np.float32)
    w_out = np.asarray(w_out, dtype=np.float32)
    b_out = np.asarray(b_out, dtype=np.float32)
    res_kernel = np.asarray(res_kernel, dtype=np.float32)
    if _NC is None:
        _NC = build_program()
    in_maps = [
        make_core_inputs(c, x, w_qkv, w_out, b_out, res_kernel) for c in range(8)
    ]
    res = run_bass_kernel_spmd(_NC, in_maps, list(range(8)), trace=False)
    out = np.zeros((4, N, DIM), np.float32)
    for c in range(8):
        out[c // 2] += res.results[c]["out"]
    return out
